# revision 1
# baseline (speedup 1.0000x reference)
"""Trainium2 Bass kernel for a Bahdanau-attention GRU decoder step.

Problem shapes (hardcoded per contract):
  x [64,1], hidden [64,1024], enc_output [64,1024,1024], W1/W2 [1024,1024],
  b1/b2 [1024], V [1024,1], bV [1], emb [32000,256],
  gru_kernel [1280,3072], gru_rkernel [1024,3072], gru_bias [2,3072],
  fc_W [1024,32000], fc_b [32000]
Returns (logits [64,32000], h [64,1024], attn [64,1024,1]) as float32.

Sharding: phase 1 is data-parallel over batch (8 rows/core); phase 2 is
tensor-parallel over the 32000 vocab (4000 cols/core) with the tiny h
gathered on the host between phases.  h0 == 0 so the GRU recurrent matmul
(gru_rkernel) vanishes; only its bias term survives.
"""

import numpy as np
import ml_dtypes

import concourse.bass as bass
import concourse.mybir as mybir
import concourse.tile as tile
from concourse import bacc
from concourse.bass import ts, ds
from concourse.bass_utils import run_bass_kernel_spmd
from concourse.masks import make_identity

F32 = mybir.dt.float32
BF16 = mybir.dt.bfloat16
AF = mybir.ActivationFunctionType
ALU = mybir.AluOpType

B, S, U, VOCAB, EMB = 64, 1024, 1024, 32000, 256
NC = 8
BC = B // NC            # 8 batch rows per core
VS = VOCAB // NC        # 4000 vocab cols per core
UC = U // 128           # 8 chunks of the 1024 unit dim
KC = (U + EMB) // 128   # 10 chunks of the GRU input dim
U3 = 3 * U
CORE_IDS = list(range(NC))
BF = ml_dtypes.bfloat16


def build_phase1():
    nc = bacc.Bacc("TRN2", target_bir_lowering=False, debug=False, num_devices=NC)

    enc = nc.dram_tensor("enc", [BC, S, U], BF16, kind="ExternalInput")
    hT = nc.dram_tensor("hT", [U, BC], BF16, kind="ExternalInput")
    w1 = nc.dram_tensor("w1", [U, U], BF16, kind="ExternalInput")
    w2 = nc.dram_tensor("w2", [U, U], BF16, kind="ExternalInput")
    v = nc.dram_tensor("v", [U, 1], BF16, kind="ExternalInput")
    qbT = nc.dram_tensor("qbT", [128, UC], F32, kind="ExternalInput")
    bv = nc.dram_tensor("bv", [128, 1], F32, kind="ExternalInput")
    xeT = nc.dram_tensor("xeT", [128, 2, BC], BF16, kind="ExternalInput")
    gk = nc.dram_tensor("gk", [U + EMB, U3], BF16, kind="ExternalInput")
    gbias = nc.dram_tensor("gbias", [1, U3], BF16, kind="ExternalInput")
    b1h = nc.dram_tensor("b1h", [BC, U], F32, kind="ExternalInput")

    h_out = nc.dram_tensor("h_out", [BC, U], F32, kind="ExternalOutput")
    attn_out = nc.dram_tensor("attn_out", [BC, S], F32, kind="ExternalOutput")

    with tile.TileContext(nc) as tc:
        with (
            tc.tile_pool(name="consts", bufs=1) as consts,
            tc.tile_pool(name="weights", bufs=1) as wpool,
            tc.tile_pool(name="encp", bufs=2) as encp,
            tc.tile_pool(name="work", bufs=2) as work,
            tc.tile_pool(name="small", bufs=2) as small,
            tc.tile_pool(name="gru", bufs=1) as grup,
            tc.tile_pool(name="pp_T", bufs=2, space="PSUM") as pp_T,
            tc.tile_pool(name="pp_tr", bufs=2, space="PSUM") as pp_tr,
            tc.tile_pool(name="pp_ctx", bufs=1, space="PSUM") as pp_ctx,
            tc.tile_pool(name="pp_sm", bufs=2, space="PSUM") as pp_sm,
        ):
            ident_bf = consts.tile([128, 128], BF16)
            make_identity(nc, ident_bf[:])
            one1x1_bf = consts.tile([1, 1], BF16)
            nc.vector.memset(one1x1_bf[:], 1.0)
            one1x1_f = consts.tile([1, 1], F32)
            nc.vector.memset(one1x1_f[:], 1.0)
            ones128_f = consts.tile([128, 1], F32)
            nc.vector.memset(ones128_f[:], 1.0)
            ones1x128_f = consts.tile([1, 128], F32)
            nc.vector.memset(ones1x128_f[:], 1.0)
            ones1x8_bf = consts.tile([1, BC], BF16)
            nc.vector.memset(ones1x8_bf[:], 1.0)

            # -------- resident weights --------
            hT_sb = wpool.tile([128, UC, BC], BF16)
            nc.sync.dma_start(hT_sb[:], hT.ap().rearrange("(c p) b -> p c b", p=128))
            w2_sb = wpool.tile([128, UC, U], BF16)
            nc.sync.dma_start(w2_sb[:], w2.ap().rearrange("(c p) n -> p c n", p=128))
            qbT_sb = wpool.tile([128, UC], F32)
            nc.sync.dma_start(qbT_sb[:], qbT.ap())
            w1_sb = wpool.tile([128, UC, U], BF16)
            nc.sync.dma_start(w1_sb[:], w1.ap().rearrange("(c p) n -> p c n", p=128))
            v_sb = wpool.tile([128, UC, 1], BF16)
            nc.sync.dma_start(v_sb[:], v.ap().rearrange("(c p) o -> p c o", p=128))
            bv_sb = wpool.tile([128, 1], F32)
            nc.sync.dma_start(bv_sb[:], bv.ap())
            xeT_sb = wpool.tile([128, 2, BC], BF16)
            nc.sync.dma_start(xeT_sb[:], xeT.ap())
            gbias_sb = wpool.tile([1, U3], BF16)
            nc.sync.dma_start(gbias_sb[:], gbias.ap())
            b1h_sb = wpool.tile([BC, U], F32)
            nc.sync.dma_start(b1h_sb[:], b1h.ap())
            gk_sb = wpool.tile([128, KC, U3], BF16)
            nc.sync.dma_start(gk_sb[:], gk.ap().rearrange("(c p) n -> p c n", p=128))

            # -------- q = hidden @ W2 + (b1 + b2), stored transposed --------
            qT_sb = wpool.tile([128, UC, BC], F32)
            for uo in range(UC):
                q_ps = pp_sm.tile([128, BC], F32, tag="sm")
                for ui in range(UC):
                    nc.tensor.matmul(
                        q_ps[:],
                        lhsT=w2_sb[:, ui, ts(uo, 128)],
                        rhs=hT_sb[:, ui, :],
                        start=(ui == 0),
                        stop=(ui == UC - 1),
                    )
                nc.scalar.activation(
                    qT_sb[:, uo, :], q_ps[:], AF.Identity,
                    bias=qbT_sb[:, ds(uo, 1)],
                )

            # -------- attention over the 8 batch rows --------
            xtT_sb = wpool.tile([128, UC, BC], BF16)  # context, transposed
            for b in range(BC):
                enc_nat = encp.tile([128, 8, U], BF16, tag="enc")
                nc.sync.dma_start(
                    enc_nat[:], enc.ap()[b].rearrange("(t p) u -> p t u", p=128)
                )

                e_col = small.tile([128, 8], F32, tag="ecol")
                e_col_bf = small.tile([128, 8], BF16, tag="ecolbf")
                ctx_ps = pp_ctx.tile([1, U], F32, tag="ctx")

                for st in range(2):
                    # transpose the 4 s-subtiles of this half into [u, s]
                    encT = work.tile([128, UC, 512], BF16, tag="encT")
                    for t4 in range(4):
                        tsub = st * 4 + t4
                        for ucx in range(UC):
                            tr = pp_tr.tile([128, 128], BF16, tag="tr")
                            nc.tensor.transpose(
                                tr[:], enc_nat[:, tsub, ts(ucx, 128)], ident_bf[:]
                            )
                            nc.vector.tensor_copy(encT[:, ucx, ts(t4, 128)], tr[:])

                    # T' = W1.T @ enc.T ; tanh(T' + q + b1) fused on ScalarE
                    tanhT = work.tile([128, UC, 512], BF16, tag="tanhT")
                    for uo in range(UC):
                        T_ps = pp_T.tile([128, 512], F32, tag="Tp")
                        for ui in range(UC):
                            nc.tensor.matmul(
                                T_ps[:],
                                lhsT=w1_sb[:, ui, ts(uo, 128)],
                                rhs=encT[:, ui, :],
                                start=(ui == 0),
                                stop=(ui == UC - 1),
                            )
                        nc.scalar.activation(
                            tanhT[:, uo, :], T_ps[:], AF.Tanh,
                            bias=qT_sb[:, uo, ds(b, 1)],
                        )

                    # score = V.T @ tanh  -> [1, 512]
                    sc_ps = pp_sm.tile([1, 512], F32, tag="sm")
                    for uo in range(UC):
                        nc.tensor.matmul(
                            sc_ps[:],
                            lhsT=v_sb[:, uo, :],
                            rhs=tanhT[:, uo, :],
                            start=(uo == 0),
                            stop=(uo == UC - 1),
                        )
                    sc_row = small.tile([1, 512], F32, tag="srow")
                    nc.scalar.copy(sc_row[:], sc_ps[:])

                    # transpose score to columns, exp(+bV)
                    scT_ps = pp_sm.tile([128, 4], F32, tag="sm")
                    for q4 in range(4):
                        nc.tensor.transpose(
                            scT_ps[:, ds(q4, 1)],
                            sc_row[0:1, ts(q4, 128)],
                            one1x1_f[:],
                        )
                    nc.scalar.activation(
                        e_col[:, ds(st * 4, 4)], scT_ps[:], AF.Exp,
                        bias=bv_sb[:, 0:1],
                    )
                    nc.vector.tensor_copy(
                        e_col_bf[:, ds(st * 4, 4)], e_col[:, ds(st * 4, 4)]
                    )

                    # unnormalized context accumulation over this half
                    for t4 in range(4):
                        tsub = st * 4 + t4
                        for n in range(2):
                            nc.tensor.matmul(
                                ctx_ps[0:1, ts(n, 512)],
                                lhsT=e_col_bf[:, ds(tsub, 1)],
                                rhs=enc_nat[:, tsub, ts(n, 512)],
                                start=(tsub == 0),
                                stop=(tsub == 7),
                                skip_group_check=True,
                            )

                # softmax denominator & normalization
                esum = small.tile([128, 1], F32, tag="esum")
                nc.vector.tensor_reduce(
                    esum[:], e_col[:], axis=mybir.AxisListType.X, op=ALU.add
                )
                den_ps = pp_sm.tile([1, 1], F32, tag="sm")
                nc.tensor.matmul(den_ps[:], lhsT=esum[:], rhs=ones128_f[:])
                inv_sb = small.tile([1, 1], F32, tag="inv")
                nc.vector.reciprocal(inv_sb[:], den_ps[:])
                invb_ps = pp_sm.tile([128, 1], F32, tag="sm")
                nc.tensor.matmul(invb_ps[:], lhsT=ones1x128_f[:], rhs=inv_sb[:])
                inv_col = small.tile([128, 1], F32, tag="invcol")
                nc.vector.tensor_copy(inv_col[:], invb_ps[:])

                attn_row = small.tile([128, 8], F32, tag="attnrow")
                nc.vector.tensor_scalar(
                    attn_row[:], e_col[:], inv_col[:, 0:1], None, ALU.mult
                )
                nc.sync.dma_start(
                    attn_out.ap()[b].rearrange("(t p) -> p t", p=128), attn_row[:]
                )

                ctx_bf = small.tile([1, U], BF16, tag="ctxbf")
                nc.vector.tensor_scalar(
                    ctx_bf[:], ctx_ps[:], inv_sb[0:1, 0:1], None, ALU.mult
                )
                for c in range(UC):
                    xt_ps = pp_sm.tile([128, 1], BF16, tag="sm")
                    nc.tensor.transpose(
                        xt_ps[:], ctx_bf[0:1, ts(c, 128)], one1x1_bf[:]
                    )
                    nc.vector.tensor_copy(xtT_sb[:, c, ds(b, 1)], xt_ps[:])

            # -------- GRU step (h0 = 0) --------
            omz = grup.tile([BC, U], F32, tag="omz")
            r_sb = grup.tile([BC, U], F32, tag="r")
            hh = grup.tile([BC, U], F32, tag="hh")
            for n in range(6):
                xi_ps = pp_sm.tile([BC, 512], F32, tag="sm")
                for kc in range(KC):
                    lhsT = xtT_sb[:, kc, :] if kc < UC else xeT_sb[:, kc - UC, :]
                    nc.tensor.matmul(
                        xi_ps[:],
                        lhsT=lhsT,
                        rhs=gk_sb[:, kc, ts(n, 512)],
                        start=(kc == 0),
                        stop=False,
                    )
                nc.tensor.matmul(
                    xi_ps[:],
                    lhsT=ones1x8_bf[:],
                    rhs=gbias_sb[0:1, ts(n, 512)],
                    start=False,
                    stop=True,
                )
                if n < 2:
                    # 1 - sigmoid(x) == sigmoid(-x)
                    nc.scalar.activation(
                        omz[:, ts(n, 512)], xi_ps[:], AF.Sigmoid, scale=-1.0
                    )
                elif n < 4:
                    nc.scalar.activation(
                        r_sb[:, ts(n - 2, 512)], xi_ps[:], AF.Sigmoid
                    )
                else:
                    sl = ts(n - 4, 512)
                    tmp = grup.tile([BC, 512], F32, tag="tmp")
                    nc.vector.tensor_tensor(
                        tmp[:], r_sb[:, sl], b1h_sb[:, sl], ALU.mult
                    )
                    tmp2 = grup.tile([BC, 512], F32, tag="tmp2")
                    nc.vector.tensor_tensor(tmp2[:], tmp[:], xi_ps[:], ALU.add)
                    nc.scalar.activation(hh[:, sl], tmp2[:], AF.Tanh)
            h_sb = grup.tile([BC, U], F32, tag="h")
            nc.vector.tensor_tensor(h_sb[:], omz[:], hh[:], ALU.mult)
            nc.sync.dma_start(h_out.ap(), h_sb[:])

    nc.compile()
    return nc


def build_phase2():
    nc = bacc.Bacc("TRN2", target_bir_lowering=False, debug=False, num_devices=NC)

    hT = nc.dram_tensor("hT", [U, B], BF16, kind="ExternalInput")
    fw = nc.dram_tensor("fw", [U, VS], BF16, kind="ExternalInput")
    fb = nc.dram_tensor("fb", [1, VS], BF16, kind="ExternalInput")
    logits = nc.dram_tensor("logits", [B, VS], F32, kind="ExternalOutput")

    nch = [512] * 7 + [VS - 7 * 512]
    with tile.TileContext(nc) as tc:
        with (
            tc.tile_pool(name="consts", bufs=1) as consts,
            tc.tile_pool(name="weights", bufs=1) as wpool,
            tc.tile_pool(name="outp", bufs=1) as outp,
            tc.tile_pool(name="ps", bufs=4, space="PSUM") as pp,
        ):
            ones1xB = consts.tile([1, B], BF16)
            nc.vector.memset(ones1xB[:], 1.0)
            hT_sb = wpool.tile([128, UC, B], BF16)
            nc.sync.dma_start(hT_sb[:], hT.ap().rearrange("(c p) b -> p c b", p=128))
            fw_sb = wpool.tile([128, UC, VS], BF16)
            nc.sync.dma_start(fw_sb[:], fw.ap().rearrange("(c p) n -> p c n", p=128))
            fb_sb = wpool.tile([1, VS], BF16)
            nc.sync.dma_start(fb_sb[:], fb.ap())

            lg_sb = outp.tile([B, VS], F32)
            n0 = 0
            for n, w in enumerate(nch):
                ps = pp.tile([B, 512], F32, tag="l")
                for kc in range(UC):
                    nc.tensor.matmul(
                        ps[:, :w],
                        lhsT=hT_sb[:, kc, :],
                        rhs=fw_sb[:, kc, ds(n0, w)],
                        start=(kc == 0),
                        stop=False,
                    )
                nc.tensor.matmul(
                    ps[:, :w], lhsT=ones1xB[:], rhs=fb_sb[0:1, ds(n0, w)],
                    start=False, stop=True,
                )
                nc.scalar.copy(lg_sb[:, ds(n0, w)], ps[:, :w])
                n0 += w
            nc.sync.dma_start(logits.ap(), lg_sb[:])

    nc.compile()
    return nc


_CACHE = {}


def _programs():
    if "p1" not in _CACHE:
        _CACHE["p1"] = build_phase1()
        _CACHE["p2"] = build_phase2()
    return _CACHE["p1"], _CACHE["p2"]


def kernel(x, hidden, enc_output, W1, b1, W2, b2, V, bV, emb,
           gru_kernel, gru_rkernel, gru_bias, fc_W, fc_b):
    x = np.asarray(x)
    hidden = np.asarray(hidden, np.float32)
    enc_output = np.asarray(enc_output, np.float32)
    W1 = np.asarray(W1, np.float32)
    b1 = np.asarray(b1, np.float32)
    W2 = np.asarray(W2, np.float32)
    b2 = np.asarray(b2, np.float32)
    V = np.asarray(V, np.float32)
    bV = np.asarray(bV, np.float32)
    emb = np.asarray(emb, np.float32)
    gru_kernel = np.asarray(gru_kernel, np.float32)
    gru_bias = np.asarray(gru_bias, np.float32)
    fc_W = np.asarray(fc_W, np.float32)
    fc_b = np.asarray(fc_b, np.float32)

    p1, p2 = _programs()

    w1_bf = W1.astype(BF)
    w2_bf = W2.astype(BF)
    v_bf = V.reshape(U, 1).astype(BF)
    qbT = np.ascontiguousarray((b1 + b2).reshape(UC, 128).T).astype(np.float32)
    bv_col = np.full((128, 1), float(bV.ravel()[0]), np.float32)
    gk_bf = gru_kernel.astype(BF)
    gbias = gru_bias[0].copy()
    gbias[: 2 * U] += gru_bias[1][: 2 * U]
    gbias_bf = gbias.reshape(1, U3).astype(BF)
    b1h = np.tile(gru_bias[1][2 * U:].reshape(1, U), (BC, 1)).astype(np.float32)
    xe = emb[x[:, 0].astype(np.int64)]  # [B, EMB] f32

    in_maps = []
    for c in range(NC):
        sl = slice(c * BC, (c + 1) * BC)
        enc_c = enc_output[sl].astype(BF)
        hT_c = np.ascontiguousarray(hidden[sl].T).astype(BF)
        xeT_c = np.ascontiguousarray(
            xe[sl].T.reshape(2, 128, BC).transpose(1, 0, 2)
        ).astype(BF)
        in_maps.append(dict(
            enc=enc_c, hT=hT_c, w1=w1_bf, w2=w2_bf, v=v_bf, qbT=qbT,
            bv=bv_col, xeT=xeT_c, gk=gk_bf, gbias=gbias_bf, b1h=b1h,
        ))
    res1 = run_bass_kernel_spmd(p1, in_maps, CORE_IDS).results
    h = np.concatenate([res1[c]["h_out"] for c in range(NC)], axis=0)
    attn = np.concatenate([res1[c]["attn_out"] for c in range(NC)], axis=0)

    hT_bf = np.ascontiguousarray(h.T).astype(BF)
    in_maps2 = []
    for c in range(NC):
        vsl = slice(c * VS, (c + 1) * VS)
        in_maps2.append(dict(
            hT=hT_bf,
            fw=np.ascontiguousarray(fc_W[:, vsl]).astype(BF),
            fb=fc_b[vsl].reshape(1, VS).astype(BF),
        ))
    res2 = run_bass_kernel_spmd(p2, in_maps2, CORE_IDS).results
    logits = np.concatenate([res2[c]["logits"] for c in range(NC)], axis=1)

    return logits.astype(np.float32), h.astype(np.float32), \
        attn.reshape(B, S, 1).astype(np.float32)


# revision 8
# speedup vs baseline: 1.0946x; 1.0946x over previous
"""Trainium2 Bass kernel for a Bahdanau-attention GRU decoder step.

Problem shapes (hardcoded per contract):
  x [64,1], hidden [64,1024], enc_output [64,1024,1024], W1/W2 [1024,1024],
  b1/b2 [1024], V [1024,1], bV [1], emb [32000,256],
  gru_kernel [1280,3072], gru_rkernel [1024,3072], gru_bias [2,3072],
  fc_W [1024,32000], fc_b [32000]
Returns (logits [64,32000], h [64,1024], attn [64,1024,1]) as float32.

Sharding: phase 1 is data-parallel over batch (8 rows/core); phase 2 is
tensor-parallel over the 32000 vocab (4000 cols/core) with the tiny h
gathered on the host between phases.  h0 == 0 so the GRU recurrent matmul
(gru_rkernel) vanishes; only its bias term survives.
"""

import numpy as np
import ml_dtypes

import concourse.bass as bass
import concourse.mybir as mybir
import concourse.tile as tile
from concourse import bacc
from concourse.bass import ts, ds
from concourse.bass_utils import run_bass_kernel_spmd
from concourse.masks import make_identity

F32 = mybir.dt.float32
BF16 = mybir.dt.bfloat16
AF = mybir.ActivationFunctionType
ALU = mybir.AluOpType

B, S, U, VOCAB, EMB = 64, 1024, 1024, 32000, 256
NC = 8
BC = B // NC            # 8 batch rows per core
VS = VOCAB // NC        # 4000 vocab cols per core
UC = U // 128           # 8 chunks of the 1024 unit dim
KC = (U + EMB) // 128   # 10 chunks of the GRU input dim
U3 = 3 * U
CORE_IDS = list(range(NC))
BF = ml_dtypes.bfloat16


def build_phase1():
    nc = bacc.Bacc("TRN2", target_bir_lowering=False, debug=False, num_devices=NC)

    enc = nc.dram_tensor("enc", [BC, S, U], BF16, kind="ExternalInput")
    hT = nc.dram_tensor("hT", [U, BC], BF16, kind="ExternalInput")
    w1 = nc.dram_tensor("w1", [U, U], BF16, kind="ExternalInput")
    w2 = nc.dram_tensor("w2", [U, U], BF16, kind="ExternalInput")
    v = nc.dram_tensor("v", [U, 1], BF16, kind="ExternalInput")
    qbT = nc.dram_tensor("qbT", [128, UC], F32, kind="ExternalInput")
    bv = nc.dram_tensor("bv", [128, 1], F32, kind="ExternalInput")
    xeT = nc.dram_tensor("xeT", [128, 2, BC], BF16, kind="ExternalInput")
    gk = nc.dram_tensor("gk", [U + EMB, U3], BF16, kind="ExternalInput")
    gbias = nc.dram_tensor("gbias", [1, U3], BF16, kind="ExternalInput")
    b1h = nc.dram_tensor("b1h", [BC, U], F32, kind="ExternalInput")

    h_out = nc.dram_tensor("h_out", [BC, U], F32, kind="ExternalOutput")
    attn_out = nc.dram_tensor("attn_out", [BC, S], F32, kind="ExternalOutput")

    with tile.TileContext(nc) as tc:
        with (
            tc.tile_pool(name="consts", bufs=1) as consts,
            tc.tile_pool(name="weights", bufs=1) as wpool,
            tc.tile_pool(name="encp", bufs=2) as encp,
            tc.tile_pool(name="work", bufs=2) as work,
            tc.tile_pool(name="small", bufs=2) as small,
            tc.tile_pool(name="gru", bufs=1) as grup,
            tc.tile_pool(name="pp_T", bufs=3, space="PSUM") as pp_T,
            tc.tile_pool(name="pp_ctx", bufs=1, space="PSUM") as pp_ctx,
            tc.tile_pool(name="pp_sm", bufs=3, space="PSUM") as pp_sm,
        ):
            one1x1_bf = consts.tile([1, 1], BF16)
            nc.vector.memset(one1x1_bf[:], 1.0)
            one1x1_f = consts.tile([1, 1], F32)
            nc.vector.memset(one1x1_f[:], 1.0)
            ones128_f = consts.tile([128, 1], F32)
            nc.vector.memset(ones128_f[:], 1.0)
            ones1x128_f = consts.tile([1, 128], F32)
            nc.vector.memset(ones1x128_f[:], 1.0)
            ones1x8_bf = consts.tile([1, BC], BF16)
            nc.vector.memset(ones1x8_bf[:], 1.0)

            # -------- resident weights --------
            hT_sb = wpool.tile([128, UC, BC], BF16)
            nc.sync.dma_start(hT_sb[:], hT.ap().rearrange("(c p) b -> p c b", p=128))
            # w2 shares the tanhT slots: it is consumed by the q matmuls
            # before the first tanhT tile is written
            w2_sb = work.tile([128, UC, U], BF16, tag="tanhT")
            nc.sync.dma_start(w2_sb[:], w2.ap().rearrange("(c p) n -> p c n", p=128))
            qbT_sb = wpool.tile([128, UC], F32)
            nc.sync.dma_start(qbT_sb[:], qbT.ap())
            w1_sb = wpool.tile([128, UC, U], BF16)
            nc.sync.dma_start(w1_sb[:], w1.ap().rearrange("(c p) n -> p c n", p=128))
            v_sb = wpool.tile([128, UC, 1], BF16)
            nc.sync.dma_start(v_sb[:], v.ap().rearrange("(c p) o -> p c o", p=128))
            bv_sb = wpool.tile([128, 1], F32)
            nc.sync.dma_start(bv_sb[:], bv.ap())
            xeT_sb = wpool.tile([128, 2, BC], BF16)
            nc.sync.dma_start(xeT_sb[:], xeT.ap())
            gbias_sb = wpool.tile([1, U3], BF16)
            nc.sync.dma_start(gbias_sb[:], gbias.ap())
            b1h_sb = wpool.tile([BC, U], F32)
            nc.sync.dma_start(b1h_sb[:], b1h.ap())
            gk_sb = wpool.tile([128, KC, U3], BF16)
            nc.sync.dma_start(gk_sb[:], gk.ap().rearrange("(c p) n -> p c n", p=128))

            # -------- q = hidden @ W2 + (b1 + b2), stored transposed --------
            qT_sb = wpool.tile([128, UC, BC], F32)
            for uo in range(UC):
                q_ps = pp_sm.tile([128, BC], F32, tag="sm")
                for ui in range(UC):
                    nc.tensor.matmul(
                        q_ps[:],
                        lhsT=w2_sb[:, ui, ts(uo, 128)],
                        rhs=hT_sb[:, ui, :],
                        start=(ui == 0),
                        stop=(ui == UC - 1),
                    )
                nc.scalar.activation(
                    qT_sb[:, uo, :], q_ps[:], AF.Identity,
                    bias=qbT_sb[:, ds(uo, 1)],
                )

            # -------- attention over the 8 batch rows --------
            xtT_sb = wpool.tile([128, UC, BC], BF16)  # context, transposed
            for b in range(BC):
                enc_nat = encp.tile([128, 8, U], BF16, tag="enc")
                nc.sync.dma_start(
                    enc_nat[:], enc.ap()[b].rearrange("(t p) u -> p t u", p=128)
                )
                e_col = small.tile([128, 8], F32, tag="ecol")
                e_col_bf = small.tile([128, 8], BF16, tag="ecolbf")
                ctx_ps = pp_ctx.tile([1, U], F32, tag="ctx")

                for st in range(2):
                    # transposed [u, s] copy straight from HBM via the DMA XBAR
                    encT = encp.tile([128, UC, 512], BF16, tag="encT")
                    for ui in range(UC):
                        nc.scalar.dma_start(
                            encT[:, ui, :],
                            enc.ap()[b][ds(st * 512, 512), ts(ui, 128)],
                            transpose=True,
                        )

                    # T' = W1.T @ enc.T ; tanh(T' + q + b1) fused on ScalarE
                    tanhT = work.tile([128, UC, 512], BF16, tag="tanhT")
                    for uo in range(UC):
                        T_ps = pp_T.tile([128, 512], F32, tag="Tp")
                        for ui in range(UC):
                            nc.tensor.matmul(
                                T_ps[:],
                                lhsT=w1_sb[:, ui, ts(uo, 128)],
                                rhs=encT[:, ui, :],
                                start=(ui == 0),
                                stop=(ui == UC - 1),
                            )
                        nc.scalar.activation(
                            tanhT[:, uo, :], T_ps[:], AF.Tanh,
                            bias=qT_sb[:, uo, ds(b, 1)],
                        )

                    # score = V.T @ tanh  -> [1, 512]
                    sc_ps = pp_sm.tile([1, 512], F32, tag="sm")
                    for uo in range(UC):
                        nc.tensor.matmul(
                            sc_ps[:],
                            lhsT=v_sb[:, uo, :],
                            rhs=tanhT[:, uo, :],
                            start=(uo == 0),
                            stop=(uo == UC - 1),
                        )
                    sc_row = small.tile([1, 512], F32, tag="srow")
                    nc.scalar.copy(sc_row[:], sc_ps[:])

                    # transpose score to columns, exp(+bV)
                    scT_ps = pp_sm.tile([128, 4], F32, tag="sm")
                    for q4 in range(4):
                        nc.tensor.transpose(
                            scT_ps[:, ds(q4, 1)],
                            sc_row[0:1, ts(q4, 128)],
                            one1x1_f[:],
                        )
                    nc.scalar.activation(
                        e_col[:, ds(st * 4, 4)], scT_ps[:], AF.Exp,
                        bias=bv_sb[:, 0:1],
                    )
                    nc.vector.tensor_copy(
                        e_col_bf[:, ds(st * 4, 4)], e_col[:, ds(st * 4, 4)]
                    )

                    # unnormalized context accumulation over this half
                    for t4 in range(4):
                        tsub = st * 4 + t4
                        for n in range(2):
                            nc.tensor.matmul(
                                ctx_ps[0:1, ts(n, 512)],
                                lhsT=e_col_bf[:, ds(tsub, 1)],
                                rhs=enc_nat[:, tsub, ts(n, 512)],
                                start=(tsub == 0),
                                stop=(tsub == 7),
                                skip_group_check=True,
                            )

                # softmax denominator & normalization
                esum = small.tile([128, 1], F32, tag="esum")
                nc.vector.tensor_reduce(
                    esum[:], e_col[:], axis=mybir.AxisListType.X, op=ALU.add
                )
                den_ps = pp_sm.tile([1, 1], F32, tag="sm")
                nc.tensor.matmul(den_ps[:], lhsT=esum[:], rhs=ones128_f[:])
                inv_sb = small.tile([1, 1], F32, tag="inv")
                nc.vector.reciprocal(inv_sb[:], den_ps[:])
                invb_ps = pp_sm.tile([128, 1], F32, tag="sm")
                nc.tensor.matmul(invb_ps[:], lhsT=ones1x128_f[:], rhs=inv_sb[:])
                inv_col = small.tile([128, 1], F32, tag="invcol")
                nc.vector.tensor_copy(inv_col[:], invb_ps[:])

                attn_row = small.tile([128, 8], F32, tag="attnrow")
                nc.vector.tensor_scalar(
                    attn_row[:], e_col[:], inv_col[:, 0:1], None, ALU.mult
                )
                nc.sync.dma_start(
                    attn_out.ap()[b].rearrange("(t p) -> p t", p=128), attn_row[:]
                )

                ctx_bf = small.tile([1, U], BF16, tag="ctxbf")
                nc.vector.tensor_scalar(
                    ctx_bf[:], ctx_ps[:], inv_sb[0:1, 0:1], None, ALU.mult
                )
                for c in range(UC):
                    xt_ps = pp_sm.tile([128, 1], BF16, tag="sm")
                    nc.tensor.transpose(
                        xt_ps[:], ctx_bf[0:1, ts(c, 128)], one1x1_bf[:]
                    )
                    nc.vector.tensor_copy(xtT_sb[:, c, ds(b, 1)], xt_ps[:])

            # -------- GRU step (h0 = 0) --------
            omz = grup.tile([BC, U], F32, tag="omz")
            r_sb = grup.tile([BC, U], F32, tag="r")
            hh = grup.tile([BC, U], F32, tag="hh")
            for n in range(6):
                xi_ps = pp_sm.tile([BC, 512], F32, tag="sm")
                for kc in range(KC):
                    lhsT = xtT_sb[:, kc, :] if kc < UC else xeT_sb[:, kc - UC, :]
                    nc.tensor.matmul(
                        xi_ps[:],
                        lhsT=lhsT,
                        rhs=gk_sb[:, kc, ts(n, 512)],
                        start=(kc == 0),
                        stop=False,
                    )
                nc.tensor.matmul(
                    xi_ps[:],
                    lhsT=ones1x8_bf[:],
                    rhs=gbias_sb[0:1, ts(n, 512)],
                    start=False,
                    stop=True,
                )
                if n < 2:
                    # 1 - sigmoid(x) == sigmoid(-x)
                    nc.scalar.activation(
                        omz[:, ts(n, 512)], xi_ps[:], AF.Sigmoid, scale=-1.0
                    )
                elif n < 4:
                    nc.scalar.activation(
                        r_sb[:, ts(n - 2, 512)], xi_ps[:], AF.Sigmoid
                    )
                else:
                    sl = ts(n - 4, 512)
                    tmp = grup.tile([BC, 512], F32, tag="tmp")
                    nc.vector.tensor_tensor(
                        tmp[:], r_sb[:, sl], b1h_sb[:, sl], ALU.mult
                    )
                    tmp2 = grup.tile([BC, 512], F32, tag="tmp2")
                    nc.vector.tensor_tensor(tmp2[:], tmp[:], xi_ps[:], ALU.add)
                    nc.scalar.activation(hh[:, sl], tmp2[:], AF.Tanh)
            h_sb = grup.tile([BC, U], F32, tag="h")
            nc.vector.tensor_tensor(h_sb[:], omz[:], hh[:], ALU.mult)
            nc.sync.dma_start(h_out.ap(), h_sb[:])

    nc.compile()
    return nc


def build_phase2():
    nc = bacc.Bacc("TRN2", target_bir_lowering=False, debug=False, num_devices=NC)

    hT = nc.dram_tensor("hT", [U, B], BF16, kind="ExternalInput")
    fw = nc.dram_tensor("fw", [U, VS], BF16, kind="ExternalInput")
    fb = nc.dram_tensor("fb", [1, VS], BF16, kind="ExternalInput")
    logits = nc.dram_tensor("logits", [B, VS], F32, kind="ExternalOutput")

    nch = [512] * 7 + [VS - 7 * 512]
    with tile.TileContext(nc) as tc:
        with (
            tc.tile_pool(name="consts", bufs=1) as consts,
            tc.tile_pool(name="weights", bufs=1) as wpool,
            tc.tile_pool(name="outp", bufs=1) as outp,
            tc.tile_pool(name="ps", bufs=1, space="PSUM") as pp,
        ):
            ones1xB = consts.tile([1, B], BF16)
            nc.vector.memset(ones1xB[:], 1.0)
            hT_sb = wpool.tile([128, UC, B], BF16)
            nc.sync.dma_start(hT_sb[:], hT.ap().rearrange("(c p) b -> p c b", p=128))
            fb_sb = wpool.tile([1, VS], BF16)
            nc.sync.dma_start(fb_sb[:], fb.ap())
            # stream fc_W one k-chunk at a time so matmuls start after the
            # first 1MB lands instead of after the whole 8MB
            fw_sb = wpool.tile([128, UC, VS], BF16)
            fw_re = fw.ap().rearrange("(c p) n -> p c n", p=128)
            for kc in range(UC):
                nc.sync.dma_start(fw_sb[:, kc, :], fw_re[:, kc, :])

            lg_sb = outp.tile([B, VS], F32)
            offs = []
            n0 = 0
            for w in nch:
                offs.append((n0, w))
                n0 += w
            pss = []
            for n, (n0, w) in enumerate(offs):
                pss.append(pp.tile([B, 512], F32, tag=f"l{n}", name=f"lgps{n}"))
            for kc in range(UC):
                for n, (n0, w) in enumerate(offs):
                    nc.tensor.matmul(
                        pss[n][:, :w],
                        lhsT=hT_sb[:, kc, :],
                        rhs=fw_sb[:, kc, ds(n0, w)],
                        start=(kc == 0),
                        stop=False,
                    )
            for n, (n0, w) in enumerate(offs):
                nc.tensor.matmul(
                    pss[n][:, :w], lhsT=ones1xB[:], rhs=fb_sb[0:1, ds(n0, w)],
                    start=False, stop=True,
                )
                nc.scalar.copy(lg_sb[:, ds(n0, w)], pss[n][:, :w])
            nc.sync.dma_start(logits.ap(), lg_sb[:])

    nc.compile()
    return nc


_CACHE = {}


def _programs():
    if "p1" not in _CACHE:
        _CACHE["p1"] = build_phase1()
        _CACHE["p2"] = build_phase2()
    return _CACHE["p1"], _CACHE["p2"]


def kernel(x, hidden, enc_output, W1, b1, W2, b2, V, bV, emb,
           gru_kernel, gru_rkernel, gru_bias, fc_W, fc_b):
    x = np.asarray(x)
    hidden = np.asarray(hidden, np.float32)
    enc_output = np.asarray(enc_output, np.float32)
    W1 = np.asarray(W1, np.float32)
    b1 = np.asarray(b1, np.float32)
    W2 = np.asarray(W2, np.float32)
    b2 = np.asarray(b2, np.float32)
    V = np.asarray(V, np.float32)
    bV = np.asarray(bV, np.float32)
    emb = np.asarray(emb, np.float32)
    gru_kernel = np.asarray(gru_kernel, np.float32)
    gru_bias = np.asarray(gru_bias, np.float32)
    fc_W = np.asarray(fc_W, np.float32)
    fc_b = np.asarray(fc_b, np.float32)

    p1, p2 = _programs()

    w1_bf = W1.astype(BF)
    w2_bf = W2.astype(BF)
    v_bf = V.reshape(U, 1).astype(BF)
    qbT = np.ascontiguousarray((b1 + b2).reshape(UC, 128).T).astype(np.float32)
    bv_col = np.full((128, 1), float(bV.ravel()[0]), np.float32)
    gk_bf = gru_kernel.astype(BF)
    gbias = gru_bias[0].copy()
    gbias[: 2 * U] += gru_bias[1][: 2 * U]
    gbias_bf = gbias.reshape(1, U3).astype(BF)
    b1h = np.tile(gru_bias[1][2 * U:].reshape(1, U), (BC, 1)).astype(np.float32)
    xe = emb[x[:, 0].astype(np.int64)]  # [B, EMB] f32

    in_maps = []
    for c in range(NC):
        sl = slice(c * BC, (c + 1) * BC)
        enc_c = enc_output[sl].astype(BF)
        hT_c = np.ascontiguousarray(hidden[sl].T).astype(BF)
        xeT_c = np.ascontiguousarray(
            xe[sl].T.reshape(2, 128, BC).transpose(1, 0, 2)
        ).astype(BF)
        in_maps.append(dict(
            enc=enc_c, hT=hT_c, w1=w1_bf, w2=w2_bf, v=v_bf, qbT=qbT,
            bv=bv_col, xeT=xeT_c, gk=gk_bf, gbias=gbias_bf, b1h=b1h,
        ))
    res1 = run_bass_kernel_spmd(p1, in_maps, CORE_IDS).results
    h = np.concatenate([res1[c]["h_out"] for c in range(NC)], axis=0)
    attn = np.concatenate([res1[c]["attn_out"] for c in range(NC)], axis=0)

    hT_bf = np.ascontiguousarray(h.T).astype(BF)
    in_maps2 = []
    for c in range(NC):
        vsl = slice(c * VS, (c + 1) * VS)
        in_maps2.append(dict(
            hT=hT_bf,
            fw=np.ascontiguousarray(fc_W[:, vsl]).astype(BF),
            fb=fc_b[vsl].reshape(1, VS).astype(BF),
        ))
    res2 = run_bass_kernel_spmd(p2, in_maps2, CORE_IDS).results
    logits = np.concatenate([res2[c]["logits"] for c in range(NC)], axis=1)

    return logits.astype(np.float32), h.astype(np.float32), \
        attn.reshape(B, S, 1).astype(np.float32)
